# revision 3
# baseline (speedup 1.0000x reference)
"""DetConB loss (nn_DetConBLoss) on 8 TRN2 NeuronCores via Bass/Tile.

Strategy (data-parallel over batch, targets replicated):
  - Host: l2-normalize preds/targets in f32, flatten to (4096, 256),
    transpose to (d, rows), cast bf16. Core c owns pred rows
    [c*512, (c+1)*512). Each core receives the full targets with columns
    rolled by c*512 so its own-image diagonal band sits at a fixed,
    compile-time-constant column range (the program is SPMD-identical).
  - Device (per core): for each of the 4 pred x target combinations,
    a (512 x 4096) bf16 matmul (K=256, fp32 PSUM accum) fused with
    exp(scale*x) on ScalarE. ACTIVATE's accum_out produces the row-sums
    of exp directly; the 128-wide diagonal band of raw logits is copied
    out for the host to extract the 16x16 own-image blocks.
  - Host: masks from the roi-index tensors (tiny O(b*n^2) work), the
    positive-pair sums, the -inf masking correction (subtract the
    exp of masked entries from the denominators), log, and the final mean.

All 34.4 GFLOP of matmul and the 67M-element exp run on device; the host
handles only O(b*n^2) = 65K-element index arithmetic.
"""
import numpy as np
import ml_dtypes

import concourse.bacc as bacc
import concourse.mybir as mybir
import concourse.tile as tile
from concourse.bass_utils import run_bass_kernel_spmd

TEMP = 0.1
EPS = 1e-11
SCALE = float(np.float32(1.0 / (TEMP + EPS)))
NCORES = 8
B, N, D = 256, 16, 256
R = B * N          # 4096 flat rows
RPC = R // NCORES  # 512 rows per core
MT = RPC // 128    # 4 row-tiles of 128 per core
BF16 = mybir.dt.bfloat16
F32 = mybir.dt.float32


def build_nc():
    """Build + schedule + compile the SPMD per-core Bass program."""
    nc = bacc.Bacc("TRN2", target_bir_lowering=False, debug=False,
                   num_devices=NCORES)

    p_dram = [nc.dram_tensor(f"p{i + 1}t", [D, RPC], BF16, kind="ExternalInput")
              for i in range(2)]
    t_dram = [nc.dram_tensor(f"t{i + 1}t", [D, R], BF16, kind="ExternalInput")
              for i in range(2)]
    sacc = nc.dram_tensor("sacc", [2, MT, 128, 4], F32, kind="ExternalOutput")
    band = nc.dram_tensor("band", [2, 2, MT, 128, 128], F32, kind="ExternalOutput")

    with tile.TileContext(nc) as tc:
        with (
            tc.tile_pool(name="const", bufs=1) as const_pool,
            tc.tile_pool(name="psum", bufs=2, space="PSUM") as psum_pool,
            tc.tile_pool(name="scratch", bufs=2) as scratch_pool,
            tc.tile_pool(name="bandp", bufs=2) as band_pool,
            tc.tile_pool(name="stripp", bufs=2) as strip_pool,
        ):
            # Persistent SBUF: targets as [K=128 partitions, kchunk*R + col],
            # preds as [128, kchunk*RPC + col].
            t_sb = [const_pool.tile([128, 2 * R], BF16, name=f"t_sb{i}", tag=f"t{i}")
                    for i in range(2)]
            p_sb = [const_pool.tile([128, 2 * RPC], BF16, name=f"p_sb{i}", tag=f"p{i}")
                    for i in range(2)]

            # Input DMAs on the sync (HWDGE) queue, ordered by first use: the
            # preds gate every matmul, then target chunks in consumption order.
            for px in range(2):
                for k in range(2):
                    nc.sync.dma_start(
                        out=p_sb[px][:, k * RPC:(k + 1) * RPC],
                        in_=p_dram[px][k * 128:(k + 1) * 128, :])
            for g in range(2):
                for tsel in range(2):
                    for k in range(2):
                        cs = g * 2048
                        nc.sync.dma_start(
                            out=t_sb[tsel][:, k * R + cs: k * R + cs + 2048],
                            in_=t_dram[tsel][k * 128:(k + 1) * 128, cs:cs + 2048])

            for px in range(2):
                for mt in range(MT):
                    strip = strip_pool.tile([128, 4], F32)
                    for tsel in range(2):
                        for g in range(2):
                            psum = psum_pool.tile([128, 2048], F32)
                            # k-outer: 4 consecutive matmuls share the same
                            # stationary weights so the PE streams back-to-back.
                            for k in range(2):
                                for j in range(4):
                                    nc.tensor.matmul(
                                        psum[:, j * 512:(j + 1) * 512],
                                        p_sb[px][:, k * RPC + mt * 128:
                                                 k * RPC + (mt + 1) * 128],
                                        t_sb[tsel][:, k * R + g * 2048 + j * 512:
                                                   k * R + g * 2048 + (j + 1) * 512],
                                        start=(k == 0), stop=(k == 1))
                            scratch = scratch_pool.tile([128, 2048], BF16)
                            nc.scalar.activation(
                                scratch, psum, mybir.ActivationFunctionType.Exp,
                                scale=SCALE)
                            nc.vector.tensor_reduce(
                                strip[:, tsel * 2 + g: tsel * 2 + g + 1], scratch,
                                axis=mybir.AxisListType.X, op=mybir.AluOpType.add)
                            if g == 0:
                                bnd = band_pool.tile([128, 128], F32)
                                nc.vector.tensor_copy(
                                    bnd, psum[:, mt * 128:(mt + 1) * 128])
                                nc.gpsimd.dma_start(out=band[px, tsel, mt], in_=bnd)
                    nc.gpsimd.dma_start(out=sacc[px, mt], in_=strip)

    nc.compile()
    return nc


_NC = None


def _get_nc():
    global _NC
    if _NC is None:
        _NC = build_nc()
    return _NC


def _l2norm(x):
    return x / np.linalg.norm(x, axis=-1, keepdims=True)


def host_prep(pred1, pred2, target1, target2):
    p1t = _l2norm(np.asarray(pred1, np.float32)).reshape(R, D).T.astype(ml_dtypes.bfloat16)
    p2t = _l2norm(np.asarray(pred2, np.float32)).reshape(R, D).T.astype(ml_dtypes.bfloat16)
    t1t = _l2norm(np.asarray(target1, np.float32)).reshape(R, D).T.astype(ml_dtypes.bfloat16)
    t2t = _l2norm(np.asarray(target2, np.float32)).reshape(R, D).T.astype(ml_dtypes.bfloat16)
    in_maps = []
    for c in range(NCORES):
        r0 = c * RPC
        in_maps.append({
            "p1t": np.ascontiguousarray(p1t[:, r0:r0 + RPC]),
            "p2t": np.ascontiguousarray(p2t[:, r0:r0 + RPC]),
            "t1t": np.ascontiguousarray(np.concatenate([t1t[:, r0:], t1t[:, :r0]], axis=1)),
            "t2t": np.ascontiguousarray(np.concatenate([t2t[:, r0:], t2t[:, :r0]], axis=1)),
        })
    return in_maps


def host_post(results, pind1, pind2, tind1, tind2):
    S = np.zeros((2, R), np.float64)
    Dg = np.zeros((2, 2, R, N), np.float32)
    for c, res in enumerate(results):
        sacc = np.asarray(res["sacc"])
        bandr = np.asarray(res["band"])
        for px in range(2):
            for mt in range(MT):
                r0 = c * RPC + mt * 128
                S[px, r0:r0 + 128] = sacc[px, mt].astype(np.float64).sum(axis=1)
                for tsel in range(2):
                    bl = bandr[px, tsel, mt]
                    for blk in range(8):
                        rr = slice(blk * 16, (blk + 1) * 16)
                        Dg[px, tsel, r0 + blk * 16: r0 + (blk + 1) * 16, :] = bl[rr, rr]
    sc = np.float32(SCALE)
    D_aa = (sc * Dg[0, 0]).reshape(B, N, N)
    D_ab = (sc * Dg[0, 1]).reshape(B, N, N)
    D_ba = (sc * Dg[1, 0]).reshape(B, N, N)
    D_bb = (sc * Dg[1, 1]).reshape(B, N, N)

    f32 = np.float32
    pind1, pind2 = np.asarray(pind1), np.asarray(pind2)
    tind1, tind2 = np.asarray(tind1), np.asarray(tind2)
    same_aa = (pind1[:, :, None] == tind1[:, None, :]).astype(f32)
    same_ab = (pind1[:, :, None] == tind2[:, None, :]).astype(f32)
    same_ba = (pind2[:, :, None] == tind1[:, None, :]).astype(f32)
    same_bb = (pind2[:, :, None] == tind2[:, None, :]).astype(f32)

    S0 = S[0].reshape(B, N)
    S1 = S[1].reshape(B, N)
    corr0 = (same_aa * np.exp(D_aa.astype(np.float64))).sum(-1)
    corr1 = (same_bb * np.exp(D_bb.astype(np.float64))).sum(-1)
    lse0 = np.log(S0 - corr0)
    lse1 = np.log(S1 - corr1)

    num_pos0 = same_ab.sum(-1)
    num_pos1 = same_ba.sum(-1)
    pos_sum0 = (same_ab * D_ab).sum(-1)
    pos_sum1 = (same_ba * D_ba).sum(-1)

    area0 = (pind1[:, :, None] == pind1[:, None, :]).astype(f32).sum(-1)
    area1 = (pind2[:, :, None] == pind2[:, None, :]).astype(f32).sum(-1)
    w0 = (num_pos0 > 0.001).astype(f32) / area0
    w1 = (num_pos1 > 0.001).astype(f32) / area1

    ce0 = -w0 * (pos_sum0 - num_pos0 * lse0) / np.maximum(num_pos0, 1.0)
    ce1 = -w1 * (pos_sum1 - num_pos1 * lse1) / np.maximum(num_pos1, 1.0)
    return np.float32(ce0.mean() + ce1.mean())


def run_hw(inputs, trace=False):
    nc = _get_nc()
    in_maps = host_prep(inputs["pred1"], inputs["pred2"],
                        inputs["target1"], inputs["target2"])
    res = run_bass_kernel_spmd(nc, in_maps, core_ids=list(range(NCORES)),
                               trace=trace)
    loss = host_post(res.results, inputs["pind1"], inputs["pind2"],
                     inputs["tind1"], inputs["tind2"])
    return loss, res


def kernel(**inputs):
    loss, _ = run_hw(inputs, trace=False)
    return loss


# revision 5
# speedup vs baseline: 1.3088x; 1.3088x over previous
"""DetConB loss (nn_DetConBLoss) on 8 TRN2 NeuronCores via Bass/Tile.

Strategy (data-parallel over batch, targets replicated):
  - Host: l2-normalize preds/targets in f32, flatten to (4096, 256),
    transpose to (d, rows), cast bf16. Core c owns pred rows
    [c*512, (c+1)*512). Each core receives the full targets with columns
    rolled by c*512 so its own-image diagonal band sits at a fixed,
    compile-time-constant column range (the program is SPMD-identical).
  - Device (per core): for each of the 4 pred x target combinations,
    a (512 x 4096) bf16 matmul (K=256, fp32 PSUM accum) fused with
    exp(scale*x) on ScalarE. ACTIVATE's accum_out produces the row-sums
    of exp directly; the 128-wide diagonal band of raw logits is copied
    out for the host to extract the 16x16 own-image blocks.
  - Host: masks from the roi-index tensors (tiny O(b*n^2) work), the
    positive-pair sums, the -inf masking correction (subtract the
    exp of masked entries from the denominators), log, and the final mean.

All 34.4 GFLOP of matmul and the 67M-element exp run on device; the host
handles only O(b*n^2) = 65K-element index arithmetic.
"""
import numpy as np
import ml_dtypes

import concourse.bacc as bacc
import concourse.mybir as mybir
import concourse.tile as tile
from concourse.bass_utils import run_bass_kernel_spmd

TEMP = 0.1
EPS = 1e-11
SCALE = float(np.float32(1.0 / (TEMP + EPS)))
NCORES = 8
B, N, D = 256, 16, 256
R = B * N          # 4096 flat rows
RPC = R // NCORES  # 512 rows per core
MT = RPC // 128    # 4 row-tiles of 128 per core
BF16 = mybir.dt.bfloat16
F32 = mybir.dt.float32


def build_nc():
    """Build + schedule + compile the SPMD per-core Bass program."""
    nc = bacc.Bacc("TRN2", target_bir_lowering=False, debug=False,
                   num_devices=NCORES)

    p_dram = [nc.dram_tensor(f"p{i + 1}t", [D, RPC], BF16, kind="ExternalInput")
              for i in range(2)]
    t_dram = [nc.dram_tensor(f"t{i + 1}t", [D, R], BF16, kind="ExternalInput")
              for i in range(2)]
    sacc = nc.dram_tensor("sacc", [2, MT, 128, 4], F32, kind="ExternalOutput")
    band = nc.dram_tensor("band", [2, 2, MT, 128, 128], F32, kind="ExternalOutput")

    with tile.TileContext(nc) as tc:
        with (
            tc.tile_pool(name="const", bufs=1) as const_pool,
            tc.tile_pool(name="psum", bufs=2, space="PSUM") as psum_pool,
            tc.tile_pool(name="scratch", bufs=2) as scratch_pool,
            tc.tile_pool(name="bandp", bufs=2) as band_pool,
            tc.tile_pool(name="stripp", bufs=2) as strip_pool,
        ):
            # Persistent SBUF: targets as [K=128 partitions, kchunk*R + col],
            # preds as [128, kchunk*RPC + col].
            t_sb = [const_pool.tile([128, 2 * R], BF16, name=f"t_sb{i}", tag=f"t{i}")
                    for i in range(2)]
            p_sb = [const_pool.tile([128, 2 * RPC], BF16, name=f"p_sb{i}", tag=f"p{i}")
                    for i in range(2)]

            # Input DMAs on the sync (HWDGE) queue, ordered by first use: the
            # preds gate every matmul, then target chunks in consumption order.
            for px in range(2):
                for k in range(2):
                    nc.sync.dma_start(
                        out=p_sb[px][:, k * RPC:(k + 1) * RPC],
                        in_=p_dram[px][k * 128:(k + 1) * 128, :])
            for g in range(2):
                for tsel in range(2):
                    for k in range(2):
                        cs = g * 2048
                        nc.sync.dma_start(
                            out=t_sb[tsel][:, k * R + cs: k * R + cs + 2048],
                            in_=t_dram[tsel][k * 128:(k + 1) * 128, cs:cs + 2048])

            for px in range(2):
                for mt in range(MT):
                    strip = strip_pool.tile([128, 4], F32)
                    nc.vector.memset(strip, 0.0)
                    for tsel in range(2):
                        # One 4096-col half = both PSUM buffers. k-outer so 8
                        # consecutive matmuls share the stationary weights and
                        # stream back-to-back (no LDWEIGHTS-induced drain).
                        ps = [psum_pool.tile([128, 2048], F32, name=f"ps{h}",
                                             tag="ps")
                              for h in range(2)]
                        for k in range(2):
                            for g in range(2):
                                for j in range(4):
                                    nc.tensor.matmul(
                                        ps[g][:, j * 512:(j + 1) * 512],
                                        p_sb[px][:, k * RPC + mt * 128:
                                                 k * RPC + (mt + 1) * 128],
                                        t_sb[tsel][:, k * R + g * 2048 + j * 512:
                                                   k * R + g * 2048 + (j + 1) * 512],
                                        start=(k == 0), stop=(k == 1))
                        for g in range(2):
                            scratch = scratch_pool.tile([128, 2048], BF16)
                            nc.scalar.activation(
                                scratch, ps[g], mybir.ActivationFunctionType.Exp,
                                scale=SCALE,
                                accum_out=strip[:, tsel * 2 + g: tsel * 2 + g + 1])
                        bnd = band_pool.tile([128, 128], F32)
                        nc.vector.tensor_copy(
                            bnd, ps[0][:, mt * 128:(mt + 1) * 128])
                        nc.gpsimd.dma_start(out=band[px, tsel, mt], in_=bnd)
                    nc.gpsimd.dma_start(out=sacc[px, mt], in_=strip)

    nc.compile()
    return nc


_NC = None


def _get_nc():
    global _NC
    if _NC is None:
        _NC = build_nc()
    return _NC


def _l2norm(x):
    return x / np.linalg.norm(x, axis=-1, keepdims=True)


def host_prep(pred1, pred2, target1, target2):
    p1t = _l2norm(np.asarray(pred1, np.float32)).reshape(R, D).T.astype(ml_dtypes.bfloat16)
    p2t = _l2norm(np.asarray(pred2, np.float32)).reshape(R, D).T.astype(ml_dtypes.bfloat16)
    t1t = _l2norm(np.asarray(target1, np.float32)).reshape(R, D).T.astype(ml_dtypes.bfloat16)
    t2t = _l2norm(np.asarray(target2, np.float32)).reshape(R, D).T.astype(ml_dtypes.bfloat16)
    in_maps = []
    for c in range(NCORES):
        r0 = c * RPC
        in_maps.append({
            "p1t": np.ascontiguousarray(p1t[:, r0:r0 + RPC]),
            "p2t": np.ascontiguousarray(p2t[:, r0:r0 + RPC]),
            "t1t": np.ascontiguousarray(np.concatenate([t1t[:, r0:], t1t[:, :r0]], axis=1)),
            "t2t": np.ascontiguousarray(np.concatenate([t2t[:, r0:], t2t[:, :r0]], axis=1)),
        })
    return in_maps


def host_post(results, pind1, pind2, tind1, tind2):
    S = np.zeros((2, R), np.float64)
    Dg = np.zeros((2, 2, R, N), np.float32)
    for c, res in enumerate(results):
        sacc = np.asarray(res["sacc"])
        bandr = np.asarray(res["band"])
        for px in range(2):
            for mt in range(MT):
                r0 = c * RPC + mt * 128
                S[px, r0:r0 + 128] = sacc[px, mt].astype(np.float64).sum(axis=1)
                for tsel in range(2):
                    bl = bandr[px, tsel, mt]
                    for blk in range(8):
                        rr = slice(blk * 16, (blk + 1) * 16)
                        Dg[px, tsel, r0 + blk * 16: r0 + (blk + 1) * 16, :] = bl[rr, rr]
    sc = np.float32(SCALE)
    D_aa = (sc * Dg[0, 0]).reshape(B, N, N)
    D_ab = (sc * Dg[0, 1]).reshape(B, N, N)
    D_ba = (sc * Dg[1, 0]).reshape(B, N, N)
    D_bb = (sc * Dg[1, 1]).reshape(B, N, N)

    f32 = np.float32
    pind1, pind2 = np.asarray(pind1), np.asarray(pind2)
    tind1, tind2 = np.asarray(tind1), np.asarray(tind2)
    same_aa = (pind1[:, :, None] == tind1[:, None, :]).astype(f32)
    same_ab = (pind1[:, :, None] == tind2[:, None, :]).astype(f32)
    same_ba = (pind2[:, :, None] == tind1[:, None, :]).astype(f32)
    same_bb = (pind2[:, :, None] == tind2[:, None, :]).astype(f32)

    S0 = S[0].reshape(B, N)
    S1 = S[1].reshape(B, N)
    corr0 = (same_aa * np.exp(D_aa.astype(np.float64))).sum(-1)
    corr1 = (same_bb * np.exp(D_bb.astype(np.float64))).sum(-1)
    lse0 = np.log(S0 - corr0)
    lse1 = np.log(S1 - corr1)

    num_pos0 = same_ab.sum(-1)
    num_pos1 = same_ba.sum(-1)
    pos_sum0 = (same_ab * D_ab).sum(-1)
    pos_sum1 = (same_ba * D_ba).sum(-1)

    area0 = (pind1[:, :, None] == pind1[:, None, :]).astype(f32).sum(-1)
    area1 = (pind2[:, :, None] == pind2[:, None, :]).astype(f32).sum(-1)
    w0 = (num_pos0 > 0.001).astype(f32) / area0
    w1 = (num_pos1 > 0.001).astype(f32) / area1

    ce0 = -w0 * (pos_sum0 - num_pos0 * lse0) / np.maximum(num_pos0, 1.0)
    ce1 = -w1 * (pos_sum1 - num_pos1 * lse1) / np.maximum(num_pos1, 1.0)
    return np.float32(ce0.mean() + ce1.mean())


def run_hw(inputs, trace=False):
    nc = _get_nc()
    in_maps = host_prep(inputs["pred1"], inputs["pred2"],
                        inputs["target1"], inputs["target2"])
    res = run_bass_kernel_spmd(nc, in_maps, core_ids=list(range(NCORES)),
                               trace=trace)
    loss = host_post(res.results, inputs["pind1"], inputs["pind2"],
                     inputs["tind1"], inputs["tind2"])
    return loss, res


def kernel(**inputs):
    loss, _ = run_hw(inputs, trace=False)
    return loss
